# revision 4
# baseline (speedup 1.0000x reference)
"""Causal depthwise conv (B=8, L=4096, D=1024, K=15) on 8 TRN2 NeuronCores.

Sharding: channels split across the 8 cores (128 channels each); every core
processes all 8 batch sequences for its channel slice. Host re-lays-out x to
[channels, batch, time] fp16 so on-chip tiles have channels on SBUF
partitions and time on the free dimension; tap shifts are free-dim offsets.

The kernel is limited by shared SBUF bandwidth (~16 B/ns/partition across
all engines), so the tap split minimizes total SBUF traffic, not just
per-engine time (fp16 compute, fp32 PSUM):
  - TensorE (10 taps {0,1,3,5,7,9,11,12,13,14}): diagonal-weight matmuls
    accumulating in 2048-wide PSUM halves. No PSUM->SBUF bridge copies.
  - ScalarE (3 taps {6,8,10}): activation-mul products (per-partition scale).
  - DVE: folds the ScalarE products (tensor_tensor, 2x packed mode), chains
    taps {2,4} as fused scalar_tensor_tensor ops (1x but only 24KB/partition
    of traffic per tap), then merges the PSUM partial directly
    (tensor_tensor with PSUM operand) - which also frees the PSUM tile.
Last batch runs a chunked epilogue to shorten the serial tail. Output is
written fp16; the host upcasts to fp32 (rel err ~5e-4 total).
"""

from contextlib import ExitStack

import numpy as np

import concourse.bacc as bacc
import concourse.tile as tile
from concourse import mybir
from concourse.bass_utils import run_bass_kernel_spmd

F32 = mybir.dt.float32
F16 = mybir.dt.float16
F16NP = np.float16

B = 8
L = 4096
D = 1024
K = 15
NCORES = 8
CPC = D // NCORES  # channels per core = 128
LP = L + K - 1  # 4110

STT_TAPS = [2, 4]  # fused product+add on DVE
SC_MUL_TAPS = [6, 8, 10]  # ScalarE products, folded on DVE
PE_TAPS = [0, 1, 3, 5, 7, 9, 11, 12, 13, 14]

_compiled_nc = None
_last_in_maps = None


def _build_nc():
    nc = bacc.Bacc(
        "TRN2",
        target_bir_lowering=False,
        debug=False,
        enable_asserts=True,
        num_devices=NCORES,
    )
    x = nc.dram_tensor("x", [CPC, B, LP], F16, kind="ExternalInput").ap()
    diag = nc.dram_tensor("diag", [len(PE_TAPS), CPC, CPC], F16, kind="ExternalInput").ap()
    w = nc.dram_tensor("w", [CPC, 16], F32, kind="ExternalInput").ap()
    out = nc.dram_tensor("out", [CPC, B, L], F16, kind="ExternalOutput").ap()

    add = mybir.AluOpType.add
    mult = mybir.AluOpType.mult

    with tile.TileContext(nc) as tc, ExitStack() as ctx:
        const_pool = ctx.enter_context(tc.tile_pool(name="const", bufs=1))
        xp = ctx.enter_context(tc.tile_pool(name="xp", bufs=3))
        prodp = ctx.enter_context(tc.tile_pool(name="prodp", bufs=5))
        sump = ctx.enter_context(tc.tile_pool(name="sump", bufs=7))
        op = ctx.enter_context(tc.tile_pool(name="op", bufs=2))
        pp = ctx.enter_context(tc.tile_pool(name="pp", bufs=2, space="PSUM"))

        wt = const_pool.tile([CPC, 16], F32, tag="w")
        nc.sync.dma_start(wt[:], w[:])
        dg = const_pool.tile([CPC, len(PE_TAPS) * CPC], F16, tag="diag")
        for j in range(len(PE_TAPS)):
            nc.sync.dma_start(dg[:, j * CPC : (j + 1) * CPC], diag[j])

        for b in range(B):
            xt = xp.tile([CPC, LP], F16, tag="x", name=f"x_{b}")
            if b == 0:
                # small first piece so the first matmuls start ASAP
                cuts = [0, 600, 2400, LP]
            else:
                cuts = [0, LP // 2, LP]
            for s0, s1 in zip(cuts[:-1], cuts[1:]):
                nc.sync.dma_start(xt[:, s0:s1], x[:, b, s0:s1])

            # ScalarE products for taps {6,8,10}
            prods = {}
            for k in SC_MUL_TAPS:
                pt = prodp.tile([CPC, L], F16, tag="prod", name=f"sp_{b}_{k}")
                nc.scalar.mul(pt[:], xt[:, k : k + L], wt[:, k : k + 1])
                prods[k] = pt

            # TensorE: 10 taps into PSUM, two 2048-wide halves
            last = b == B - 1
            pss = []
            for h in range(2):
                t0 = h * 2048
                ps = pp.tile([CPC, 2048], F32, tag="ps", name=f"ps_{b}_{h}")
                pss.append(ps)
                for ji, k in enumerate(PE_TAPS):
                    for q in range(4):
                        nc.tensor.matmul(
                            ps[:, q * 512 : (q + 1) * 512],
                            dg[:, ji * CPC : (ji + 1) * CPC],
                            xt[:, t0 + k + q * 512 : t0 + k + (q + 1) * 512],
                            start=(ji == 0),
                            stop=(ji == len(PE_TAPS) - 1),
                        )

            # DVE: fold Sc products, chain STT taps, merge PSUM
            f1 = sump.tile([CPC, L], F16, tag="sum", name=f"f1_{b}")
            nc.vector.tensor_tensor(f1[:], prods[8][:], prods[10][:], add)
            f2 = sump.tile([CPC, L], F16, tag="sum", name=f"f2_{b}")
            nc.vector.tensor_tensor(f2[:], f1[:], prods[6][:], add)
            c0 = sump.tile([CPC, L], F16, tag="sum", name=f"c0_{b}")
            nc.vector.scalar_tensor_tensor(
                c0[:], xt[:, 2 : 2 + L], wt[:, 2:3], f2[:], mult, add
            )
            c1 = sump.tile([CPC, L], F16, tag="sum", name=f"c1_{b}")
            nc.vector.scalar_tensor_tensor(
                c1[:], xt[:, 4 : 4 + L], wt[:, 4:5], c0[:], mult, add
            )
            ot = op.tile([CPC, L], F16, tag="osb", name=f"o_{b}")
            if last:
                for c in range(4):
                    h, q = divmod(c, 2)
                    sl = slice(c * 1024, (c + 1) * 1024)
                    nc.vector.tensor_tensor(
                        ot[:, sl], c1[:, sl], pss[h][:, q * 1024 : (q + 1) * 1024], add
                    )
                    nc.scalar.dma_start(out[:, b, sl], ot[:, sl])
            else:
                for h in range(2):
                    sl = slice(h * 2048, (h + 1) * 2048)
                    nc.vector.tensor_tensor(ot[:, sl], c1[:, sl], pss[h][:], add)
                nc.scalar.dma_start(out[:, b, :], ot[:])

    nc.compile()
    return nc


def kernel(x: np.ndarray, weight: np.ndarray) -> np.ndarray:
    """x: [8, 4096, 1024] fp32, weight: [15, 1, 1024] fp32 ->
    [8, 4096, 1024] fp32 causal depthwise conv."""
    global _compiled_nc
    if _compiled_nc is None:
        _compiled_nc = _build_nc()
    nc = _compiled_nc

    x = np.ascontiguousarray(x, dtype=np.float32)
    wk = np.ascontiguousarray(weight, dtype=np.float32).reshape(K, D)
    x16 = x.astype(F16NP)
    wk16 = wk.astype(F16NP)

    in_maps = []
    for c in range(NCORES):
        sl = slice(c * CPC, (c + 1) * CPC)
        xpad = np.zeros((CPC, B, LP), dtype=F16NP)
        xpad[:, :, K - 1 :] = x16[:, :, sl].transpose(2, 0, 1)
        dgc = np.zeros((len(PE_TAPS), CPC, CPC), dtype=F16NP)
        didx = np.arange(CPC)
        for j, k in enumerate(PE_TAPS):
            dgc[j, didx, didx] = wk16[k, sl]
        wt = np.zeros((CPC, 16), dtype=np.float32)
        wt[:, :K] = wk[:, sl].T
        in_maps.append({"x": xpad, "diag": dgc, "w": wt})

    global _last_in_maps
    _last_in_maps = in_maps
    res = run_bass_kernel_spmd(nc, in_maps, list(range(NCORES)))

    out = np.empty((B, L, D), dtype=np.float32)
    for c in range(NCORES):
        sl = slice(c * CPC, (c + 1) * CPC)
        out[:, :, sl] = res.results[c]["out"].transpose(1, 2, 0).astype(np.float32)
    return out


# revision 6
# speedup vs baseline: 1.0426x; 1.0426x over previous
"""Causal depthwise conv (B=8, L=4096, D=1024, K=15) on 8 TRN2 NeuronCores.

Sharding: channels split across the 8 cores (128 channels each); every core
processes all 8 batch sequences for its channel slice. Host re-lays-out x to
[channels, batch, time] fp16 so on-chip tiles have channels on SBUF
partitions and time on the free dimension; tap shifts are free-dim offsets.

Engine split of the 15 taps (fp16 compute, fp32 PSUM), shaped by two
constraints measured on HW: PE matmul cadence is ~240ns per FD-512 matmul
when the other engines are streaming (shared-SBUF contention), and DVE
semaphore overhead scales with op count, so DVE runs few, wide ops:
  - TensorE (9 taps {0,1,3,5,7,9,11,13,14}): diagonal-weight matmuls into
    2048-wide PSUM halves; ScalarE bridges PSUM->SBUF fp16 (decouples DVE
    from the PE tail).
  - ScalarE (3 taps {6,8,10}): activation-mul products, plus the bridges.
  - DVE (taps {2,4} + all accumulation): two tensor_tensor folds of the
    ScalarE products, a two-op scalar_tensor_tensor chain for taps {2,4},
    one tensor_tensor merge with the bridged PE partial. 5 wide ops/batch.
Last batch runs a chunked epilogue to shorten the serial tail. Output is
written fp16; the host upcasts to fp32 (rel err ~5e-4 total).
"""

from contextlib import ExitStack

import numpy as np

import concourse.bacc as bacc
import concourse.tile as tile
from concourse import mybir
from concourse.bass_utils import run_bass_kernel_spmd

F32 = mybir.dt.float32
F16 = mybir.dt.float16
F16NP = np.float16

B = 8
L = 4096
D = 1024
K = 15
NCORES = 8
CPC = D // NCORES  # channels per core = 128
LP = L + K - 1  # 4110

STT_TAPS = [2, 4]
SC_MUL_TAPS = [6, 8, 10]
PE_TAPS = [0, 1, 3, 5, 7, 9, 11, 12, 13, 14]

_compiled_nc = None
_last_in_maps = None


def _maybe_enable_ldw_opt():
    """Flip walrus --enable-ldw-opt when BASS_LDW_OPT=1 (A/B experiment)."""
    import os

    if os.environ.get("BASS_LDW_OPT") != "1":
        return
    import concourse.bass_utils as bu

    if getattr(bu, "_ldw_patched", False):
        return
    orig = bu.run_command

    def patched(cmd, *a, **kw):
        if isinstance(cmd, list):
            cmd = [
                "--enable-ldw-opt=true" if c == "--enable-ldw-opt=false" else c
                for c in cmd
            ]
        return orig(cmd, *a, **kw)

    bu.run_command = patched
    bu._ldw_patched = True


_maybe_enable_ldw_opt()



def _build_nc():
    nc = bacc.Bacc(
        "TRN2",
        target_bir_lowering=False,
        debug=False,
        enable_asserts=True,
        num_devices=NCORES,
    )
    x = nc.dram_tensor("x", [CPC, B, LP], F16, kind="ExternalInput").ap()
    diag = nc.dram_tensor("diag", [len(PE_TAPS), CPC, CPC], F16, kind="ExternalInput").ap()
    w = nc.dram_tensor("w", [CPC, 16], F32, kind="ExternalInput").ap()
    out = nc.dram_tensor("out", [CPC, B, L], F16, kind="ExternalOutput").ap()

    add = mybir.AluOpType.add
    mult = mybir.AluOpType.mult

    with tile.TileContext(nc) as tc, ExitStack() as ctx:
        const_pool = ctx.enter_context(tc.tile_pool(name="const", bufs=1))
        xp = ctx.enter_context(tc.tile_pool(name="xp", bufs=3))
        prodp = ctx.enter_context(tc.tile_pool(name="prodp", bufs=5))
        sump = ctx.enter_context(tc.tile_pool(name="sump", bufs=7))
        accp = ctx.enter_context(tc.tile_pool(name="accp", bufs=2))
        op = ctx.enter_context(tc.tile_pool(name="op", bufs=2))
        pp = ctx.enter_context(tc.tile_pool(name="pp", bufs=2, space="PSUM"))

        wt = const_pool.tile([CPC, 16], F32, tag="w")
        nc.sync.dma_start(wt[:], w[:])
        dg = const_pool.tile([CPC, len(PE_TAPS) * CPC], F16, tag="diag")
        for j in range(len(PE_TAPS)):
            nc.sync.dma_start(dg[:, j * CPC : (j + 1) * CPC], diag[j])

        for b in range(B):
            xt = xp.tile([CPC, LP], F16, tag="x", name=f"x_{b}")
            if b == 0:
                # small first piece so the first matmuls start ASAP
                cuts = [0, 600, 1300, 2100, 2900, LP]
            else:
                cuts = [0, LP // 2, LP]
            for s0, s1 in zip(cuts[:-1], cuts[1:]):
                nc.sync.dma_start(xt[:, s0:s1], x[:, b, s0:s1])

            # ScalarE products for taps {6,8,10}
            prods = {}
            for k in SC_MUL_TAPS:
                pt = prodp.tile([CPC, L], F16, tag="prod", name=f"sp_{b}_{k}")
                nc.scalar.mul(pt[:], xt[:, k : k + L], wt[:, k : k + 1])
                prods[k] = pt

            # TensorE: 9 taps into PSUM, two 2048-wide halves + ScalarE bridge
            last = b == B - 1
            acc = accp.tile([CPC, L], F16, tag="acc", name=f"acc_{b}")
            for h in range(2):
                t0 = h * 2048
                ps = pp.tile([CPC, 2048], F32, tag="ps", name=f"ps_{b}_{h}")
                for ji, k in enumerate(PE_TAPS):
                    for q in range(4):
                        nc.tensor.matmul(
                            ps[:, q * 512 : (q + 1) * 512],
                            dg[:, ji * CPC : (ji + 1) * CPC],
                            xt[:, t0 + k + q * 512 : t0 + k + (q + 1) * 512],
                            start=(ji == 0),
                            stop=(ji == len(PE_TAPS) - 1),
                        )
                if last:
                    for q in range(2):
                        nc.scalar.copy(
                            acc[:, t0 + q * 1024 : t0 + (q + 1) * 1024],
                            ps[:, q * 1024 : (q + 1) * 1024],
                        )
                else:
                    nc.scalar.copy(acc[:, t0 : t0 + 2048], ps[:])

            # DVE: fold products, STT chain for {2,4}, merge with bridge
            f1 = sump.tile([CPC, L], F16, tag="sum", name=f"f1_{b}")
            nc.vector.tensor_tensor(f1[:], prods[6][:], prods[8][:], add)
            f2 = sump.tile([CPC, L], F16, tag="sum", name=f"f2_{b}")
            nc.vector.tensor_tensor(f2[:], f1[:], prods[10][:], add)
            c0 = sump.tile([CPC, L], F16, tag="sum", name=f"c0_{b}")
            nc.vector.scalar_tensor_tensor(
                c0[:], xt[:, 2 : 2 + L], wt[:, 2:3], f2[:], mult, add
            )
            c1 = sump.tile([CPC, L], F16, tag="sum", name=f"c1_{b}")
            nc.vector.scalar_tensor_tensor(
                c1[:], xt[:, 4 : 4 + L], wt[:, 4:5], c0[:], mult, add
            )
            ot = op.tile([CPC, L], F16, tag="osb", name=f"o_{b}")
            if last:
                for c in range(4):
                    sl = slice(c * 1024, (c + 1) * 1024)
                    nc.vector.tensor_tensor(ot[:, sl], c1[:, sl], acc[:, sl], add)
                    nc.scalar.dma_start(out[:, b, sl], ot[:, sl])
            else:
                nc.vector.tensor_tensor(ot[:], c1[:], acc[:], add)
                nc.scalar.dma_start(out[:, b, :], ot[:])

    nc.compile()
    return nc


def kernel(x: np.ndarray, weight: np.ndarray) -> np.ndarray:
    """x: [8, 4096, 1024] fp32, weight: [15, 1, 1024] fp32 ->
    [8, 4096, 1024] fp32 causal depthwise conv."""
    global _compiled_nc
    if _compiled_nc is None:
        _compiled_nc = _build_nc()
    nc = _compiled_nc

    x = np.ascontiguousarray(x, dtype=np.float32)
    wk = np.ascontiguousarray(weight, dtype=np.float32).reshape(K, D)
    x16 = x.astype(F16NP)
    wk16 = wk.astype(F16NP)

    in_maps = []
    for c in range(NCORES):
        sl = slice(c * CPC, (c + 1) * CPC)
        xpad = np.zeros((CPC, B, LP), dtype=F16NP)
        xpad[:, :, K - 1 :] = x16[:, :, sl].transpose(2, 0, 1)
        dgc = np.zeros((len(PE_TAPS), CPC, CPC), dtype=F16NP)
        didx = np.arange(CPC)
        for j, k in enumerate(PE_TAPS):
            dgc[j, didx, didx] = wk16[k, sl]
        wt = np.zeros((CPC, 16), dtype=np.float32)
        wt[:, :K] = wk[:, sl].T
        in_maps.append({"x": xpad, "diag": dgc, "w": wt})

    global _last_in_maps
    _last_in_maps = in_maps
    res = run_bass_kernel_spmd(nc, in_maps, list(range(NCORES)))

    out = np.empty((B, L, D), dtype=np.float32)
    for c in range(NCORES):
        sl = slice(c * CPC, (c + 1) * CPC)
        out[:, :, sl] = res.results[c]["out"].transpose(1, 2, 0).astype(np.float32)
    return out


# revision 7
# speedup vs baseline: 1.0867x; 1.0423x over previous
"""Causal depthwise conv (B=8, L=4096, D=1024, K=15) on 8 TRN2 NeuronCores.

Sharding: channels split across the 8 cores (128 channels each); every core
processes all 8 batch sequences for its channel slice. Host re-lays-out x to
[channels, batch, time] fp16 so on-chip tiles have channels on SBUF
partitions and time on the free dimension; tap shifts are free-dim offsets.

Engine split of the 15 taps (fp16 compute, fp32 PSUM), shaped by two
constraints measured on HW: PE matmul cadence is ~240ns per FD-512 matmul
when the other engines are streaming (shared-SBUF contention), and DVE
semaphore overhead scales with op count, so DVE runs few, wide ops:
  - TensorE (9 taps {0,1,3,5,7,9,11,13,14}): diagonal-weight matmuls into
    2048-wide PSUM halves; ScalarE bridges PSUM->SBUF fp16 (decouples DVE
    from the PE tail).
  - ScalarE (3 taps {6,8,10}): activation-mul products, plus the bridges.
  - DVE (taps {2,4} + all accumulation): two tensor_tensor folds of the
    ScalarE products, a two-op scalar_tensor_tensor chain for taps {2,4},
    one tensor_tensor merge with the bridged PE partial. 5 wide ops/batch.
Last batch runs a chunked epilogue to shorten the serial tail. Output is
written fp16; the host upcasts to fp32 (rel err ~5e-4 total).
"""

from contextlib import ExitStack

import numpy as np

import concourse.bacc as bacc
import concourse.tile as tile
from concourse import mybir
from concourse.bass_utils import run_bass_kernel_spmd

F32 = mybir.dt.float32
F16 = mybir.dt.float16
F16NP = np.float16

B = 8
L = 4096
D = 1024
K = 15
NCORES = 8
CPC = D // NCORES  # channels per core = 128
LP = L + K - 1  # 4110

import os

KCFG = os.environ.get("KCFG", "v5")
STT_TAPS = [2, 4]
SC_MUL_TAPS = [6, 8, 10]
PE_TAPS = [0, 1, 3, 5, 7, 9, 11, 12, 13, 14]

_compiled_nc = None
_last_in_maps = None


def _maybe_enable_ldw_opt():
    """Flip walrus --enable-ldw-opt when BASS_LDW_OPT=1 (A/B experiment)."""
    import os

    if os.environ.get("BASS_LDW_OPT") != "1":
        return
    import concourse.bass_utils as bu

    if getattr(bu, "_ldw_patched", False):
        return
    orig = bu.run_command

    def patched(cmd, *a, **kw):
        if isinstance(cmd, list):
            cmd = [
                "--enable-ldw-opt=true" if c == "--enable-ldw-opt=false" else c
                for c in cmd
            ]
        return orig(cmd, *a, **kw)

    bu.run_command = patched
    bu._ldw_patched = True


_maybe_enable_ldw_opt()



def _build_nc():
    nc = bacc.Bacc(
        "TRN2",
        target_bir_lowering=False,
        debug=False,
        enable_asserts=True,
        num_devices=NCORES,
    )
    x = nc.dram_tensor("x", [CPC, B, LP], F16, kind="ExternalInput").ap()
    diag = nc.dram_tensor("diag", [len(PE_TAPS), CPC, CPC], F16, kind="ExternalInput").ap()
    w = nc.dram_tensor("w", [CPC, 16], F32, kind="ExternalInput").ap()
    out = nc.dram_tensor("out", [CPC, B, L], F16, kind="ExternalOutput").ap()

    add = mybir.AluOpType.add
    mult = mybir.AluOpType.mult

    with tile.TileContext(nc) as tc, ExitStack() as ctx:
        const_pool = ctx.enter_context(tc.tile_pool(name="const", bufs=1))
        xp = ctx.enter_context(tc.tile_pool(name="xp", bufs=3))
        prodp = ctx.enter_context(tc.tile_pool(name="prodp", bufs=5))
        sump = ctx.enter_context(tc.tile_pool(name="sump", bufs=7))
        accp = ctx.enter_context(tc.tile_pool(name="accp", bufs=2))
        op = ctx.enter_context(tc.tile_pool(name="op", bufs=2))
        pp = ctx.enter_context(tc.tile_pool(name="pp", bufs=2, space="PSUM"))

        wt = const_pool.tile([CPC, 16], F32, tag="w")
        nc.sync.dma_start(wt[:], w[:])
        dg = const_pool.tile([CPC, len(PE_TAPS) * CPC], F16, tag="diag")
        for j in range(len(PE_TAPS)):
            nc.sync.dma_start(dg[:, j * CPC : (j + 1) * CPC], diag[j])

        for b in range(B):
            xt = xp.tile([CPC, LP], F16, tag="x", name=f"x_{b}")
            if b == 0:
                # small first piece so the first matmuls start ASAP
                cuts = [0, 600, 1300, 2100, 2900, LP]
            else:
                cuts = [0, LP // 2, LP]
            for s0, s1 in zip(cuts[:-1], cuts[1:]):
                nc.sync.dma_start(xt[:, s0:s1], x[:, b, s0:s1])

            # ScalarE products for taps {6,8,10}
            prods = {}
            for k in SC_MUL_TAPS:
                pt = prodp.tile([CPC, L], F16, tag="prod", name=f"sp_{b}_{k}")
                nc.scalar.mul(pt[:], xt[:, k : k + L], wt[:, k : k + 1])
                prods[k] = pt

            # TensorE: 9 taps into PSUM, two 2048-wide halves + ScalarE bridge
            last = b == B - 1
            acc = accp.tile([CPC, L], F16, tag="acc", name=f"acc_{b}")
            for h in range(2):
                t0 = h * 2048
                ps = pp.tile([CPC, 2048], F32, tag="ps", name=f"ps_{b}_{h}")
                for ji, k in enumerate(PE_TAPS):
                    for q in range(4):
                        nc.tensor.matmul(
                            ps[:, q * 512 : (q + 1) * 512],
                            dg[:, ji * CPC : (ji + 1) * CPC],
                            xt[:, t0 + k + q * 512 : t0 + k + (q + 1) * 512],
                            start=(ji == 0),
                            stop=(ji == len(PE_TAPS) - 1),
                        )
                if last:
                    for q in range(2):
                        nc.scalar.copy(
                            acc[:, t0 + q * 1024 : t0 + (q + 1) * 1024],
                            ps[:, q * 1024 : (q + 1) * 1024],
                        )
                else:
                    nc.scalar.copy(acc[:, t0 : t0 + 2048], ps[:])

            # DVE: accumulate taps {2,4} and the Sc products
            if KCFG == "v2":
                # TS muls for {2,4} (4x mode) + fold chain
                for k in STT_TAPS:
                    pt = prodp.tile([CPC, L], F16, tag="prod", name=f"dp_{b}_{k}")
                    nc.vector.tensor_scalar_mul(
                        pt[:], xt[:, k : k + L], wt[:, k : k + 1]
                    )
                    prods[k] = pt
                s = prods[2]
                for i, k in enumerate([4, 6, 8, 10]):
                    dst = sump.tile([CPC, L], F16, tag="sum", name=f"s_{b}_{i}")
                    nc.vector.tensor_tensor(dst[:], prods[k][:], s[:], add)
                    s = dst
                c1 = s
            else:
                f1 = sump.tile([CPC, L], F16, tag="sum", name=f"f1_{b}")
                nc.vector.tensor_tensor(f1[:], prods[6][:], prods[8][:], add)
                f2 = sump.tile([CPC, L], F16, tag="sum", name=f"f2_{b}")
                nc.vector.tensor_tensor(f2[:], f1[:], prods[10][:], add)
                c0 = sump.tile([CPC, L], F16, tag="sum", name=f"c0_{b}")
                nc.vector.scalar_tensor_tensor(
                    c0[:], xt[:, 2 : 2 + L], wt[:, 2:3], f2[:], mult, add
                )
                c1 = sump.tile([CPC, L], F16, tag="sum", name=f"c1_{b}")
                nc.vector.scalar_tensor_tensor(
                    c1[:], xt[:, 4 : 4 + L], wt[:, 4:5], c0[:], mult, add
                )
            ot = op.tile([CPC, L], F16, tag="osb", name=f"o_{b}")
            if last:
                for c in range(4):
                    sl = slice(c * 1024, (c + 1) * 1024)
                    nc.vector.tensor_tensor(ot[:, sl], c1[:, sl], acc[:, sl], add)
                    nc.scalar.dma_start(out[:, b, sl], ot[:, sl])
            else:
                nc.vector.tensor_tensor(ot[:], c1[:], acc[:], add)
                nc.scalar.dma_start(out[:, b, :], ot[:])

    nc.compile()
    return nc


def kernel(x: np.ndarray, weight: np.ndarray) -> np.ndarray:
    """x: [8, 4096, 1024] fp32, weight: [15, 1, 1024] fp32 ->
    [8, 4096, 1024] fp32 causal depthwise conv."""
    global _compiled_nc
    if _compiled_nc is None:
        _compiled_nc = _build_nc()
    nc = _compiled_nc

    x = np.ascontiguousarray(x, dtype=np.float32)
    wk = np.ascontiguousarray(weight, dtype=np.float32).reshape(K, D)
    x16 = x.astype(F16NP)
    wk16 = wk.astype(F16NP)

    in_maps = []
    for c in range(NCORES):
        sl = slice(c * CPC, (c + 1) * CPC)
        xpad = np.zeros((CPC, B, LP), dtype=F16NP)
        xpad[:, :, K - 1 :] = x16[:, :, sl].transpose(2, 0, 1)
        dgc = np.zeros((len(PE_TAPS), CPC, CPC), dtype=F16NP)
        didx = np.arange(CPC)
        for j, k in enumerate(PE_TAPS):
            dgc[j, didx, didx] = wk16[k, sl]
        wt = np.zeros((CPC, 16), dtype=np.float32)
        wt[:, :K] = wk[:, sl].T
        in_maps.append({"x": xpad, "diag": dgc, "w": wt})

    global _last_in_maps
    _last_in_maps = in_maps
    res = run_bass_kernel_spmd(nc, in_maps, list(range(NCORES)))

    out = np.empty((B, L, D), dtype=np.float32)
    for c in range(NCORES):
        sl = slice(c * CPC, (c + 1) * CPC)
        out[:, :, sl] = res.results[c]["out"].transpose(1, 2, 0).astype(np.float32)
    return out


# revision 8
# speedup vs baseline: 1.1157x; 1.0267x over previous
"""Causal depthwise conv (B=8, L=4096, D=1024, K=15) on 8 TRN2 NeuronCores.

Sharding: channels split across the 8 cores (128 channels each); every core
processes all 8 batch sequences for its channel slice. Host re-lays-out x to
[channels, batch, time] fp16 so on-chip tiles have channels on SBUF
partitions and time on the free dimension; tap shifts are free-dim offsets.

Engine split of the 15 taps (fp16 compute, fp32 PSUM), shaped by two
constraints measured on HW: PE matmul cadence is ~240ns per FD-512 matmul
when the other engines are streaming (shared-SBUF contention), and DVE
semaphore overhead scales with op count, so DVE runs few, wide ops:
  - TensorE (9 taps {0,1,3,5,7,9,11,13,14}): diagonal-weight matmuls into
    2048-wide PSUM halves; ScalarE bridges PSUM->SBUF fp16 (decouples DVE
    from the PE tail).
  - ScalarE (3 taps {6,8,10}): activation-mul products, plus the bridges.
  - DVE (taps {2,4} + all accumulation): two tensor_tensor folds of the
    ScalarE products, a two-op scalar_tensor_tensor chain for taps {2,4},
    one tensor_tensor merge with the bridged PE partial. 5 wide ops/batch.
Last batch runs a chunked epilogue to shorten the serial tail. Output is
written fp16; the host upcasts to fp32 (rel err ~5e-4 total).
"""

from contextlib import ExitStack

import numpy as np

import concourse.bacc as bacc
import concourse.tile as tile
from concourse import mybir
from concourse.bass_utils import run_bass_kernel_spmd

F32 = mybir.dt.float32
F16 = mybir.dt.float16
F16NP = np.float16

B = 8
L = 4096
D = 1024
K = 15
NCORES = 8
CPC = D // NCORES  # channels per core = 128
LP = L + K - 1  # 4110

import os

KCFG = os.environ.get("KCFG", "v2")
STT_TAPS = [2, 4]
SC_MUL_TAPS = [6, 8, 10]
PE_TAPS = [0, 1, 3, 5, 7, 9, 11, 12, 13, 14]

_compiled_nc = None
_last_in_maps = None


def _maybe_enable_ldw_opt():
    """Flip walrus --enable-ldw-opt when BASS_LDW_OPT=1 (A/B experiment)."""
    import os

    if os.environ.get("BASS_LDW_OPT") != "1":
        return
    import concourse.bass_utils as bu

    if getattr(bu, "_ldw_patched", False):
        return
    orig = bu.run_command

    def patched(cmd, *a, **kw):
        if isinstance(cmd, list):
            cmd = [
                "--enable-ldw-opt=true" if c == "--enable-ldw-opt=false" else c
                for c in cmd
            ]
        return orig(cmd, *a, **kw)

    bu.run_command = patched
    bu._ldw_patched = True


_maybe_enable_ldw_opt()



def _build_nc():
    nc = bacc.Bacc(
        "TRN2",
        target_bir_lowering=False,
        debug=False,
        enable_asserts=True,
        num_devices=NCORES,
    )
    x = nc.dram_tensor("x", [CPC, B, LP], F16, kind="ExternalInput").ap()
    diag = nc.dram_tensor("diag", [len(PE_TAPS), CPC, CPC], F16, kind="ExternalInput").ap()
    w = nc.dram_tensor("w", [CPC, 16], F32, kind="ExternalInput").ap()
    out = nc.dram_tensor("out", [CPC, B, L], F16, kind="ExternalOutput").ap()

    add = mybir.AluOpType.add
    mult = mybir.AluOpType.mult

    with tile.TileContext(nc) as tc, ExitStack() as ctx:
        const_pool = ctx.enter_context(tc.tile_pool(name="const", bufs=1))
        xp = ctx.enter_context(tc.tile_pool(name="xp", bufs=4))
        prodp = ctx.enter_context(tc.tile_pool(name="prodp", bufs=5))
        sump = ctx.enter_context(tc.tile_pool(name="sump", bufs=7))
        accp = ctx.enter_context(tc.tile_pool(name="accp", bufs=2))
        op = ctx.enter_context(tc.tile_pool(name="op", bufs=2))
        pp = ctx.enter_context(tc.tile_pool(name="pp", bufs=2, space="PSUM"))

        # x(b0) loads first: sync ring takes the front (PE-critical),
        # scalar ring takes the tail in parallel.
        xt0 = xp.tile([CPC, LP], F16, tag="x", name="x_0")
        for s0, s1 in [(0, 600), (600, 1300), (1300, 2100)]:
            nc.sync.dma_start(xt0[:, s0:s1], x[:, 0, s0:s1])
        nc.scalar.dma_start(xt0[:, 2900:LP], x[:, 0, 2900:LP])
        nc.sync.dma_start(xt0[:, 2100:2900], x[:, 0, 2100:2900])

        wt = const_pool.tile([CPC, 16], F32, tag="w")
        nc.sync.dma_start(wt[:], w[:])
        dg = const_pool.tile([CPC, len(PE_TAPS) * CPC], F16, tag="diag")
        for j in range(len(PE_TAPS)):
            nc.sync.dma_start(dg[:, j * CPC : (j + 1) * CPC], diag[j])

        for b in range(B):
            if b == 0:
                xt = xt0
            else:
                xt = xp.tile([CPC, LP], F16, tag="x", name=f"x_{b}")
                for s0, s1 in [(0, LP // 2), (LP // 2, LP)]:
                    nc.sync.dma_start(xt[:, s0:s1], x[:, b, s0:s1])

            # ScalarE products for taps {6,8,10}
            prods = {}
            for k in SC_MUL_TAPS:
                pt = prodp.tile([CPC, L], F16, tag="prod", name=f"sp_{b}_{k}")
                nc.scalar.mul(pt[:], xt[:, k : k + L], wt[:, k : k + 1])
                prods[k] = pt

            # TensorE: 9 taps into PSUM, two 2048-wide halves + ScalarE bridge
            last = b == B - 1
            acc = accp.tile([CPC, L], F16, tag="acc", name=f"acc_{b}")
            for h in range(2):
                t0 = h * 2048
                ps = pp.tile([CPC, 2048], F32, tag="ps", name=f"ps_{b}_{h}")
                for ji, k in enumerate(PE_TAPS):
                    for q in range(4):
                        nc.tensor.matmul(
                            ps[:, q * 512 : (q + 1) * 512],
                            dg[:, ji * CPC : (ji + 1) * CPC],
                            xt[:, t0 + k + q * 512 : t0 + k + (q + 1) * 512],
                            start=(ji == 0),
                            stop=(ji == len(PE_TAPS) - 1),
                        )
                if last:
                    for q in range(2):
                        nc.scalar.copy(
                            acc[:, t0 + q * 1024 : t0 + (q + 1) * 1024],
                            ps[:, q * 1024 : (q + 1) * 1024],
                        )
                else:
                    nc.scalar.copy(acc[:, t0 : t0 + 2048], ps[:])

            # DVE: accumulate taps {2,4} and the Sc products
            if KCFG == "v2":
                # TS muls for {2,4} (4x mode) + fold chain
                for k in STT_TAPS:
                    pt = prodp.tile([CPC, L], F16, tag="prod", name=f"dp_{b}_{k}")
                    nc.vector.tensor_scalar_mul(
                        pt[:], xt[:, k : k + L], wt[:, k : k + 1]
                    )
                    prods[k] = pt
                s = prods[2]
                for i, k in enumerate([4, 6, 8, 10]):
                    dst = sump.tile([CPC, L], F16, tag="sum", name=f"s_{b}_{i}")
                    nc.vector.tensor_tensor(dst[:], prods[k][:], s[:], add)
                    s = dst
                c1 = s
            else:
                f1 = sump.tile([CPC, L], F16, tag="sum", name=f"f1_{b}")
                nc.vector.tensor_tensor(f1[:], prods[6][:], prods[8][:], add)
                f2 = sump.tile([CPC, L], F16, tag="sum", name=f"f2_{b}")
                nc.vector.tensor_tensor(f2[:], f1[:], prods[10][:], add)
                c0 = sump.tile([CPC, L], F16, tag="sum", name=f"c0_{b}")
                nc.vector.scalar_tensor_tensor(
                    c0[:], xt[:, 2 : 2 + L], wt[:, 2:3], f2[:], mult, add
                )
                c1 = sump.tile([CPC, L], F16, tag="sum", name=f"c1_{b}")
                nc.vector.scalar_tensor_tensor(
                    c1[:], xt[:, 4 : 4 + L], wt[:, 4:5], c0[:], mult, add
                )
            ot = op.tile([CPC, L], F16, tag="osb", name=f"o_{b}")
            if last:
                for c in range(4):
                    sl = slice(c * 1024, (c + 1) * 1024)
                    nc.vector.tensor_tensor(ot[:, sl], c1[:, sl], acc[:, sl], add)
                    nc.scalar.dma_start(out[:, b, sl], ot[:, sl])
            else:
                nc.vector.tensor_tensor(ot[:], c1[:], acc[:], add)
                nc.scalar.dma_start(out[:, b, :], ot[:])

    nc.compile()
    return nc


def kernel(x: np.ndarray, weight: np.ndarray) -> np.ndarray:
    """x: [8, 4096, 1024] fp32, weight: [15, 1, 1024] fp32 ->
    [8, 4096, 1024] fp32 causal depthwise conv."""
    global _compiled_nc
    if _compiled_nc is None:
        _compiled_nc = _build_nc()
    nc = _compiled_nc

    x = np.ascontiguousarray(x, dtype=np.float32)
    wk = np.ascontiguousarray(weight, dtype=np.float32).reshape(K, D)
    x16 = x.astype(F16NP)
    wk16 = wk.astype(F16NP)

    in_maps = []
    for c in range(NCORES):
        sl = slice(c * CPC, (c + 1) * CPC)
        xpad = np.zeros((CPC, B, LP), dtype=F16NP)
        xpad[:, :, K - 1 :] = x16[:, :, sl].transpose(2, 0, 1)
        dgc = np.zeros((len(PE_TAPS), CPC, CPC), dtype=F16NP)
        didx = np.arange(CPC)
        for j, k in enumerate(PE_TAPS):
            dgc[j, didx, didx] = wk16[k, sl]
        wt = np.zeros((CPC, 16), dtype=np.float32)
        wt[:, :K] = wk[:, sl].T
        in_maps.append({"x": xpad, "diag": dgc, "w": wt})

    global _last_in_maps
    _last_in_maps = in_maps
    res = run_bass_kernel_spmd(nc, in_maps, list(range(NCORES)))

    out = np.empty((B, L, D), dtype=np.float32)
    for c in range(NCORES):
        sl = slice(c * CPC, (c + 1) * CPC)
        out[:, :, sl] = res.results[c]["out"].transpose(1, 2, 0).astype(np.float32)
    return out
